# revision 18
# baseline (speedup 1.0000x reference)
"""MoDGPT (topk_masking) Trainium2 kernel.

Math note (verified against the reference to ~2e-6 rel err): in the
reference's _mod_step, processed == routed, so
new_vals = routed*w + (1-w)*routed == routed up to ~1 ulp — every MoD
step is an identity on x.  The three ACT recurrence states are therefore
all == x, and the whole module collapses to

    t   = x . halt_w                        (per token)
    h   = sigmoid(t + halt_b)
    c   = p1 + p2 + p3   (ACT halting recurrence driven only by h)
    out = c * x
    ponder = 0.01 * mean(n_upd)

which is one read + one write of x — a pure memory-bound kernel.

Sharding: data-parallel over tokens; 32768 tokens / 8 cores = 4096
tokens (16 MiB) per core; halt_w/halt_b replicated.
"""

import numpy as np

P = 128          # SBUF partitions
D = 1024         # model dim
G = 4            # tokens per partition per big tile (big tile = P*G tokens)
N_CORES = 8
B, S = 4, 8192   # full input shape (B, S, D)
TOK_PER_CORE = (B * S) // N_CORES   # 4096
THRESH = 1.0 - 0.01                 # 1 - EPS
PONDER = 0.01
ACT_STEPS = 3

_nc_cache = {}
last_results = None  # BassKernelResults of the most recent kernel() call


def _build_nc(tokens_per_core=TOK_PER_CORE, g=G, bufs=6, repeats=1,
              compute=True, store_eng="sync", subset_nbt=None):
    import concourse.bacc as bacc
    import concourse.mybir as mybir
    from concourse import tile

    F32 = mybir.dt.float32
    Alu = mybir.AluOpType
    Act = mybir.ActivationFunctionType

    nbt = tokens_per_core // (P * g)          # big tiles per core
    assert nbt * P * g == tokens_per_core
    fd = g * D                                # free dim of a big tile

    nc = bacc.Bacc("TRN2", target_bir_lowering=False, debug=False)
    store = {"sync": nc.sync, "scalar": nc.scalar, "gpsimd": nc.gpsimd}[store_eng]

    x_in = nc.dram_tensor("x", (tokens_per_core, D), F32, kind="ExternalInput")
    w_in = nc.dram_tensor("halt_w", (D,), F32, kind="ExternalInput")
    b_in = nc.dram_tensor("halt_b", (1,), F32, kind="ExternalInput")
    y_out = nc.dram_tensor("y", (tokens_per_core, D), F32, kind="ExternalOutput")
    nsum_out = nc.dram_tensor("nsum", (P, 1), F32, kind="ExternalOutput")

    # token index = i*(P*g) + p*g + q  → per-partition rows are contiguous
    x_t = x_in.rearrange("(n p q) d -> n p (q d)", p=P, q=g)
    y_t = y_out.rearrange("(n p q) d -> n p (q d)", p=P, q=g)

    with tile.TileContext(nc) as tc:
        with (
            tc.tile_pool(name="const", bufs=1) as cpool,
            tc.tile_pool(name="data", bufs=bufs) as dpool,
            tc.tile_pool(name="scratch", bufs=2) as spool,
            tc.tile_pool(name="stat", bufs=2) as stpool,
        ):
            # broadcast halt_w / halt_b across all 128 partitions
            wrow = cpool.tile([1, D], F32)
            brow = cpool.tile([1, 1], F32)
            nc.sync.dma_start(out=wrow, in_=w_in[None, :])
            nc.sync.dma_start(out=brow, in_=b_in[None, :])
            wb = cpool.tile([P, D], F32)
            bb = cpool.tile([P, 1], F32)
            nc.gpsimd.partition_broadcast(wb, wrow[0:1, :])
            nc.gpsimd.partition_broadcast(bb, brow[0:1, :])

            nacc = cpool.tile([P, 1], F32)
            nc.vector.memset(nacc, 0.0)

            loop_nbt = nbt if subset_nbt is None else subset_nbt
            for i in [i for _ in range(repeats) for i in range(loop_nbt)]:
                xt = dpool.tile([P, fd], F32, tag="xt")
                nc.sync.dma_start(out=xt, in_=x_t[i])

                if not compute:
                    store.dma_start(out=y_t[i], in_=xt)
                    continue

                tcol = stpool.tile([P, g], F32, tag="tcol")
                for q in range(g):
                    scr = spool.tile([P, D], F32, tag="scr")
                    nc.vector.tensor_mul(scr, xt[:, q * D:(q + 1) * D], wb)
                    # free-dim sum via ScalarE accumulate (keeps DVE free)
                    nc.scalar.activation(
                        scr, scr, Act.Copy, accum_out=tcol[:, q:q + 1]
                    )

                # h = sigmoid(t + b)
                h = stpool.tile([P, g], F32, tag="h")
                nc.scalar.activation(h, tcol, Act.Sigmoid, bias=bb[:, 0:1], scale=1.0)

                # ACT halting recurrence (all (P, g) elementwise)
                hc = stpool.tile([P, g], F32, tag="hc")      # 1 - h
                nc.vector.tensor_scalar(hc, h, -1.0, 1.0, Alu.mult, Alu.add)
                # step 1:  p1 = h + ur1*(1-h);  ur1 = h > thresh
                ur = stpool.tile([P, g], F32, tag="ur")
                nc.vector.tensor_scalar(ur, h, THRESH, None, Alu.is_gt)
                tmp = stpool.tile([P, g], F32, tag="tmp")
                nc.vector.tensor_mul(tmp, ur, hc)
                c = stpool.tile([P, g], F32, tag="c")
                nc.vector.tensor_add(c, h, tmp)              # c = p1 (for now)
                acc = stpool.tile([P, g], F32, tag="acc")
                nc.vector.tensor_copy(acc, c)                # acc = p1
                rem = stpool.tile([P, g], F32, tag="rem")
                nc.vector.tensor_scalar(rem, c, -1.0, 1.0, Alu.mult, Alu.add)
                nt = stpool.tile([P, g], F32, tag="nt")
                nc.vector.memset(nt, 1.0)

                for _step in range(1, ACT_STEPS):
                    still = stpool.tile([P, g], F32, tag="still")
                    nc.vector.tensor_scalar(still, acc, THRESH, None, Alu.is_lt)
                    hs = stpool.tile([P, g], F32, tag="hs")
                    nc.vector.tensor_mul(hs, h, still)
                    na = stpool.tile([P, g], F32, tag="na")
                    nc.vector.tensor_add(na, acc, hs)
                    urr = stpool.tile([P, g], F32, tag="urr")
                    nc.vector.tensor_scalar(urr, na, THRESH, None, Alu.is_gt)
                    ur2 = stpool.tile([P, g], F32, tag="ur2")
                    nc.vector.tensor_mul(ur2, urr, still)
                    uh = stpool.tile([P, g], F32, tag="uh")
                    nc.vector.tensor_sub(uh, still, ur2)
                    pa = stpool.tile([P, g], F32, tag="pa")
                    nc.vector.tensor_mul(pa, uh, h)
                    pb = stpool.tile([P, g], F32, tag="pb")
                    nc.vector.tensor_mul(pb, ur2, rem)
                    p = stpool.tile([P, g], F32, tag="p")
                    nc.vector.tensor_add(p, pa, pb)
                    nc.vector.tensor_add(acc, acc, p)
                    nc.vector.tensor_sub(rem, rem, p)
                    nc.vector.tensor_add(c, c, p)
                    nc.vector.tensor_add(nt, nt, still)

                # ponder partial sums
                ncol = stpool.tile([P, 1], F32, tag="ncol")
                nc.vector.reduce_sum(ncol, nt, axis=mybir.AxisListType.X)
                nc.vector.tensor_add(nacc, nacc, ncol)

                # out = c * x  (per-partition-scalar scale, 2x fp32 on DVE), in place
                for q in range(g):
                    nc.vector.tensor_scalar_mul(
                        xt[:, q * D:(q + 1) * D],
                        xt[:, q * D:(q + 1) * D],
                        c[:, q:q + 1],
                    )
                store.dma_start(out=y_t[i], in_=xt)

            nc.sync.dma_start(out=nsum_out[:, :], in_=nacc)

    nc.compile()
    return nc


def get_nc(**kw):
    key = tuple(sorted(kw.items()))
    if key not in _nc_cache:
        _nc_cache[key] = _build_nc(**kw)
    return _nc_cache[key]


def kernel(x, router_w, halt_w, halt_b):
    from concourse.bass_utils import run_bass_kernel_spmd

    x = np.ascontiguousarray(np.asarray(x, dtype=np.float32))
    halt_w = np.asarray(halt_w, dtype=np.float32)
    halt_b = np.asarray(halt_b, dtype=np.float32)
    b, s, d = x.shape
    tok = b * s
    tpc = tok // N_CORES
    x_flat = x.reshape(tok, d)

    nc = get_nc()
    in_maps = [
        {
            "x": x_flat[c * tpc:(c + 1) * tpc],
            "halt_w": halt_w,
            "halt_b": halt_b,
        }
        for c in range(N_CORES)
    ]
    global last_results
    last_results = run_bass_kernel_spmd(nc, in_maps, list(range(N_CORES)))
    res = last_results.results

    out = np.empty((tok, d), dtype=np.float32)
    total_n = 0.0
    for c in range(N_CORES):
        out[c * tpc:(c + 1) * tpc] = res[c]["y"]
        total_n += float(res[c]["nsum"].sum())
    out = out.reshape(b, s, d)
    ponder = np.float32(
        np.float32(PONDER) * np.float32(np.float32(total_n) / np.float32(tok))
    )
    return out, ponder
